# revision 4
# baseline (speedup 1.0000x reference)
"""Bidirectional LSTM on 8 trn2 NeuronCores.

Sharding: 2 directions x 4-way batch split (B_local=8 per core). Every core
runs the IDENTICAL forward-scan program; backward cores receive
time-reversed x and their outputs are re-reversed on the host.

Per-core plan (B=8, T=512, I=256, H=512, G=4H=2048):
  1. PE-transpose W_ih/W_hh into fp16 stationary tiles (lhsT layout).
  2. Precompute xp = x @ W_ih.T + b for all T into an SBUF-resident fp16
     buffer (gates.T layout).
  3. 512-step scan: per gate-group, 16 matmuls [128x128]x[128x8] accumulate
     gates.T in a dedicated PSUM bank; DVE adds xp; ACT sigmoid/tanh; DVE
     cell update; h kept fp16 for the next step's moving operand.
  4. Output written windowed in hardware-native layout; unscrambled on host.
"""

import numpy as np

B_FULL, T, I, H = 32, 512, 256, 512
G = 4 * H
N_CORES = 8
B = B_FULL // 4          # per-core batch
KH = H // 128            # 4 k-chunks for W_hh
KI = I // 128            # 2 k-chunks for W_ih
M = G // 128             # 16 m-chunks (4 per gate)
WIN = 16                 # scan steps per output DMA window
T_SCAN = T

_BUILT = {}


def _install_tile_patch():
    """This container's walrus accepts only ONE sync-wait per CTRL-class
    instruction (Drain/NoOp). Tile's kernel-tail drain aggregates one wait
    per semaphore lane onto a single Drain -> split them one per drain."""
    import bass_rust
    import concourse.tile as tile

    if getattr(tile.TileContext, "_drain_split_patched", False):
        return

    def _patched_dab(self, tick_clock, wait_clock):
        from concourse.tile import ScopedClock

        nc = self.nc
        drain_inst = nc.sync.drain()
        wait_clock.add_sem_waits(
            drain_inst.ins, ScopedClock({None: tick_clock.global_clock})
        )
        si = drain_inst.ins.sync_info
        waits = list(si.on_wait) if si is not None else []
        if len(waits) > 1:
            si.on_wait = waits[:1]
            for w in waits[1:]:
                d2 = nc.sync.drain()
                si2 = d2.ins.sync_info
                if si2 is None:
                    d2.ins.sync_info = bass_rust.SyncInfo(on_wait=[w], on_update=[])
                else:
                    si2.on_wait = list(si2.on_wait) + [w]
        nc.all_engine_barrier()
        assert self.sems is not None
        popped = nc._tile_sem_poison_stack.pop()
        assert popped is self._sem_poison
        nc.clear_and_free_semaphores(list(self.sems.allocated().values()))
        nc.all_engine_barrier()

    tile.TileContext._drain_and_barrier = _patched_dab
    tile.TileContext._drain_split_patched = True

    # This walrus build accepts at most ONE sync-wait per instruction (any
    # opcode). Split every multi-wait instruction at BIR-JSON level into
    # single-wait NoOps followed by the real instruction with one wait.
    import json
    import concourse.bass as bass

    if getattr(bass.Bass, "_json_wait_split_patched", False):
        return
    _orig_tjb = bass.Bass.to_json_bytes

    def _split_json(self):
        raw = _orig_tjb(self)
        m = json.loads(raw)
        ctr = 0
        changed = False
        for fn in m.get("functions", []):
            for bb in fn.get("blocks", []):
                out = []
                for inst in bb.get("instructions", []):
                    si = inst.get("sync_info")
                    waits = (si or {}).get("on_wait") or []
                    if len(waits) > 1:
                        changed = True
                        for w in waits[:-1]:
                            ctr += 1
                            nop = {
                                "engine": inst["engine"],
                                "ins": [],
                                "outs": [],
                                "name": f"WSPLIT-{ctr}",
                                "opcode": "NoOp",
                                "sync_info": {"on_update": [], "on_wait": [w]},
                            }
                            if "debug" in inst:
                                nop["debug"] = inst["debug"]
                            out.append(nop)
                        si["on_wait"] = [waits[-1]]
                    out.append(inst)
                bb["instructions"] = out
        if not changed:
            return raw
        return json.dumps(m).encode()

    bass.Bass.to_json_bytes = _split_json
    bass.Bass._json_wait_split_patched = True


def _build(t_scan):
    import concourse.bass as bass
    import concourse.tile as tile
    from concourse import masks, mybir
    from contextlib import ExitStack

    _install_tile_patch()
    f32 = mybir.dt.float32
    f16 = mybir.dt.float16

    nc = bass.Bass()
    x_d = nc.dram_tensor("x", [B, T, I], f32, kind="ExternalInput")
    wih_d = nc.dram_tensor("W_ih", [G, I], f32, kind="ExternalInput")
    whh_d = nc.dram_tensor("W_hh", [G, H], f32, kind="ExternalInput")
    b_d = nc.dram_tensor("b", [G], f32, kind="ExternalInput")
    n_win = (t_scan + WIN - 1) // WIN
    out_d = nc.dram_tensor("out_raw", [n_win, 128, WIN * 4 * B], f32,
                           kind="ExternalOutput")

    TB = B * T  # 4096 flattened (b, t) columns, b-major

    with tile.TileContext(nc) as tc, ExitStack() as ctx:
        sig = mybir.ActivationFunctionType.Sigmoid
        tanh = mybir.ActivationFunctionType.Tanh

        wpool = ctx.enter_context(tc.tile_pool(name="w", bufs=1))
        whhT = wpool.tile([128, KH * M * 128], f16)   # tile (k,m) at (k*M+m)*128
        wihT = wpool.tile([128, KI * M * 128], f16)
        xT = wpool.tile([128, KI * TB], f16)          # k-chunk ki at ki*TB
        xp = wpool.tile([128, M * TB], f16)           # chunk m at m*TB, col b*T+t
        b_sb = wpool.tile([128, M], f32)
        ident = wpool.tile([128, 128], f32)
        masks.make_identity(nc, ident[:])
        nc.gpsimd.dma_start(b_sb[:], b_d.rearrange("(m p) -> p m", p=128))

        # ---- phase A+B: transpose weights and x into lhsT/moving layouts ----
        with tc.tile_pool(name="stage", bufs=3) as spool, \
             tc.tile_pool(name="tpsum", bufs=4, space="PSUM") as tpool:
            for m in range(M):
                st = spool.tile([128, H], f32, tag="stw")
                nc.gpsimd.dma_start(st[:], whh_d[m * 128:(m + 1) * 128, :])
                for k in range(KH):
                    ps = tpool.tile([128, 128], f32, tag="tp")
                    nc.tensor.transpose(ps[:], st[:, k * 128:(k + 1) * 128], ident[:])
                    dst = whhT[:, (k * M + m) * 128:(k * M + m + 1) * 128]
                    if (m + k) % 2 == 0:
                        nc.vector.tensor_copy(dst, ps[:])
                    else:
                        nc.scalar.copy(dst, ps[:])
            for m in range(M):
                st = spool.tile([128, I], f32, tag="stw")
                nc.gpsimd.dma_start(st[:], wih_d[m * 128:(m + 1) * 128, :])
                for k in range(KI):
                    ps = tpool.tile([128, 128], f32, tag="tp")
                    nc.tensor.transpose(ps[:], st[:, k * 128:(k + 1) * 128], ident[:])
                    dst = wihT[:, (k * M + m) * 128:(k * M + m + 1) * 128]
                    if (m + k) % 2 == 0:
                        nc.vector.tensor_copy(dst, ps[:])
                    else:
                        nc.scalar.copy(dst, ps[:])
            x_flat = x_d.rearrange("b t i -> (b t) i")
            for c in range(TB // 128):
                st = spool.tile([128, I], f32, tag="stw")
                nc.gpsimd.dma_start(st[:], x_flat[c * 128:(c + 1) * 128, :])
                for k in range(KI):
                    ps = tpool.tile([128, 128], f32, tag="tp")
                    nc.tensor.transpose(ps[:], st[:, k * 128:(k + 1) * 128], ident[:])
                    dst = xT[:, k * TB + c * 128:k * TB + (c + 1) * 128]
                    if (c + k) % 2 == 0:
                        nc.vector.tensor_copy(dst, ps[:])
                    else:
                        nc.scalar.copy(dst, ps[:])

        # ---- phase C: xp = x @ W_ih.T + b, fp16, gates.T layout ----
        NXP = 512
        with tc.tile_pool(name="xppsum", bufs=4, space="PSUM") as xpp:
            for m in range(M):
                for n in range(TB // NXP):
                    ps = xpp.tile([128, NXP], f32, tag="xps")
                    for k in range(KI):
                        nc.tensor.matmul(
                            ps[:],
                            wihT[:, (k * M + m) * 128:(k * M + m + 1) * 128],
                            xT[:, k * TB + n * NXP:k * TB + (n + 1) * NXP],
                            start=(k == 0), stop=(k == KI - 1),
                        )
                    dst = xp[:, m * TB + n * NXP:m * TB + (n + 1) * NXP]
                    if n % 2 == 0:
                        nc.vector.tensor_scalar_add(dst, ps[:], b_sb[:, m:m + 1])
                    else:
                        nc.scalar.add(dst, ps[:], b_sb[:, m:m + 1])

        # ---- phase D: the scan ----
        # col layout of h/c/gate tiles: 8k + b  (k = H 128-chunk, b = batch)
        xp4 = xp.rearrange("p (m b t) -> p m b t", m=M, b=B)
        with tc.tile_pool(name="gpsum", bufs=2, space="PSUM") as gp, \
             tc.tile_pool(name="acts", bufs=2) as apool, \
             tc.tile_pool(name="state", bufs=2) as stp, \
             tc.tile_pool(name="outb", bufs=2) as obp:
            h_prev = stp.tile([128, KH * B], f16, tag="h")
            c_prev = stp.tile([128, KH * B], f32, tag="c")
            nc.vector.memset(h_prev[:], 0.0)
            nc.vector.memset(c_prev[:], 0.0)

            GATE_ORDER = [(2, tanh), (0, sig), (1, sig), (3, sig)]  # g, i, f, o
            ob = None
            for t in range(t_scan):
                s = t % WIN
                if s == 0:
                    ob = obp.tile([128, WIN * KH * B], f32, tag="ob")
                acts = {}
                for g, func in GATE_ORDER:
                    ps = gp.tile([128, KH * B], f32, tag=f"ps{g}")
                    for mi in range(KH):
                        m = 4 * g + mi
                        for k in range(KH):
                            nc.tensor.matmul(
                                ps[:, 8 * mi:8 * mi + 8],
                                whhT[:, (k * M + m) * 128:(k * M + m + 1) * 128],
                                h_prev[:, 8 * k:8 * k + 8],
                                start=(k == 0), stop=(k == KH - 1),
                            )
                    sg = apool.tile([128, KH * B], f32, tag=f"sg{g}")
                    nc.vector.tensor_add(
                        sg.rearrange("p (m b) -> p m b", m=KH),
                        ps.rearrange("p (m b) -> p m b", m=KH),
                        xp4[:, 4 * g:4 * g + 4, :, t],
                    )
                    ac = apool.tile([128, KH * B], f32, tag=f"ac{g}")
                    nc.scalar.activation(ac[:], sg[:], func)
                    acts[g] = ac
                ig = apool.tile([128, KH * B], f32, tag="ig")
                nc.vector.tensor_mul(ig[:], acts[0][:], acts[2][:])
                fc = apool.tile([128, KH * B], f32, tag="fc")
                nc.vector.tensor_mul(fc[:], acts[1][:], c_prev[:])
                c_new = stp.tile([128, KH * B], f32, tag="c")
                nc.vector.tensor_add(c_new[:], ig[:], fc[:])
                th = apool.tile([128, KH * B], f32, tag="th")
                nc.scalar.activation(th[:], c_new[:], tanh)
                h_new = stp.tile([128, KH * B], f16, tag="h")
                nc.vector.tensor_mul(h_new[:], acts[3][:], th[:])
                nc.vector.tensor_mul(ob[:, 32 * s:32 * s + 32], acts[3][:], th[:])
                h_prev, c_prev = h_new, c_new
                if s == WIN - 1 or t == t_scan - 1:
                    nc.gpsimd.dma_start(out_d[t // WIN], ob[:])

    return nc


def _get_nc(t_scan):
    key = t_scan
    if key not in _BUILT:
        _BUILT[key] = _build(t_scan)
    return _BUILT[key]


def kernel(x, W_ih_f, W_hh_f, b_f, W_ih_b, W_hh_b, b_b, _t_scan=T_SCAN,
           _profile=False):
    from concourse.bass_utils import run_bass_kernel_spmd

    x = np.asarray(x, dtype=np.float32)
    params = {
        0: (np.asarray(W_ih_f, np.float32), np.asarray(W_hh_f, np.float32),
            np.asarray(b_f, np.float32)),
        1: (np.asarray(W_ih_b, np.float32), np.asarray(W_hh_b, np.float32),
            np.asarray(b_b, np.float32)),
    }
    in_maps = []
    for c in range(N_CORES):
        d = c // 4          # 0 = forward, 1 = backward
        bs = (c % 4) * B
        xs = x[bs:bs + B]
        if d == 1:
            xs = xs[:, ::-1]
        wih, whh, bb = params[d]
        in_maps.append({
            "x": np.ascontiguousarray(xs),
            "W_ih": wih, "W_hh": whh, "b": bb,
        })

    nc = _get_nc(_t_scan)
    kw = {}
    if _profile:
        import tempfile
        kw = dict(trace=True, tmpdir=tempfile.mkdtemp(prefix="lstm_prof_"))
    res = run_bass_kernel_spmd(nc, in_maps, list(range(N_CORES)), **kw)
    if _profile:
        print(f"HW exec time: {res.exec_time_ns} ns")
        kernel._last_profile = res
        kernel._last_tmpdir = kw.get("tmpdir")

    n_win = (_t_scan + WIN - 1) // WIN
    t_out = n_win * WIN
    halves = []
    for d in range(2):
        parts = []
        for c4 in range(4):
            raw = np.asarray(res.results[d * 4 + c4]["out_raw"])
            # raw[w, p, 32s + 8k + b] = h[b, 16w+s, 128k+p]
            h = raw.reshape(n_win, 128, WIN, KH, B)
            h = h.transpose(4, 0, 2, 3, 1).reshape(B, t_out, H)[:, :_t_scan]
            parts.append(h)
        hcat = np.concatenate(parts, axis=0)
        if d == 1:
            hcat = hcat[:, ::-1]
        halves.append(hcat)
    return np.concatenate(halves, axis=2).astype(np.float32)
